# revision 4
# baseline (speedup 1.0000x reference)
"""Multi-head attention kernel for Trainium2, SPMD across 8 NeuronCores.

Problem: q,k,v [B=2, H=16, S=2048, D=64] f32;
  out = softmax(q @ k^T / sqrt(4)) @ v      (scale quirk: d_k = tensor RANK = 4)

Sharding: 32 (b,h) heads split 4-per-core across 8 cores; forward is fully
data-parallel (no collectives).

Per-core algorithm (flash-attention style, scores kept transposed):
  - Build Q^T, K^T [64, S] in SBUF via TensorE transposes of natural
    [128, 64] tiles (f32 has no DMA-xbar transpose path).
  - For each q-chunk (1024 q): for each t-tile (128 t):
      S^T[t, q] = K^T_tile.T @ Q^T          (matmul, contraction d=64)
      P^T = exp(0.5 * S^T)                  (ScalarE, PSUM -> SBUF, FD=1024)
      O^T[d+1, q] += V1_tile.T @ P^T        (matmul, contraction t=128;
                                             V1 = [V | ones] so row 64 of O^T
                                             accumulates the softmax denom)
  - Epilogue: transpose O^T back to [q, 65] via TensorE, multiply by
    reciprocal(denom) on VectorE, DMA out.

No max-subtraction in the softmax: scaled scores are ~N(0, 4); |s| < ~25 for
these inputs, exp stays well inside f32 range.
"""

import numpy as np

B, H, S, D = 2, 16, 2048, 64
N_CORES = 8
HPC = (B * H) // N_CORES  # heads per core = 4
P = 128
T_TILES = S // P  # 16
QCHUNK = 1024
N_QCHUNKS = S // QCHUNK  # 2
SCALE = 0.5  # 1/sqrt(d_k) with d_k = k.ndim = 4 (faithful to reference)

_CACHE = {}


def _build_nc():
    from contextlib import ExitStack

    import concourse.bacc as bacc
    import concourse.mybir as mybir
    import concourse.tile as tile
    from concourse.masks import make_identity

    fp32 = mybir.dt.float32
    Exp = mybir.ActivationFunctionType.Exp

    nc = bacc.Bacc()
    q_ext = nc.declare_dram_parameter("q", [HPC, S, D], fp32, isOutput=False)
    k_ext = nc.declare_dram_parameter("k", [HPC, S, D], fp32, isOutput=False)
    v_ext = nc.declare_dram_parameter("v", [HPC, S, D], fp32, isOutput=False)
    out_ext = nc.declare_dram_parameter("out", [HPC, S, D], fp32, isOutput=True)

    with ExitStack() as ctx:
        tc = ctx.enter_context(tile.TileContext(nc))
        consts = ctx.enter_context(tc.tile_pool(name="consts", bufs=1))
        identity = consts.tile([P, P], fp32)
        make_identity(nc, identity)

        nat = ctx.enter_context(tc.tile_pool(name="nat", bufs=2))
        vpool = ctx.enter_context(tc.tile_pool(name="vpool", bufs=2))
        qkt = ctx.enter_context(tc.tile_pool(name="qkt", bufs=2))
        ptp = ctx.enter_context(tc.tile_pool(name="ptp", bufs=3))
        otp = ctx.enter_context(tc.tile_pool(name="otp", bufs=2))
        op = ctx.enter_context(tc.tile_pool(name="op", bufs=2))
        rp = ctx.enter_context(tc.tile_pool(name="rp", bufs=2))
        # PSUM budget (8 banks of 2KB/partition):
        #   scores [128,1024] x2 bufs = 4 banks, O-acc [65,1024] = 2 banks,
        #   qk-transpose staging 1 bank, epilogue-transpose staging 1 bank.
        ps_s = ctx.enter_context(tc.tile_pool(name="ps_s", bufs=2, space="PSUM"))
        ps_o = ctx.enter_context(tc.tile_pool(name="ps_o", bufs=1, space="PSUM"))
        ps_t = ctx.enter_context(tc.tile_pool(name="ps_t", bufs=1, space="PSUM"))
        ps_e = ctx.enter_context(tc.tile_pool(name="ps_e", bufs=1, space="PSUM"))

        for h in range(HPC):
            qn = nat.tile([P, T_TILES, D], fp32, tag="qn")
            nc.sync.dma_start(out=qn, in_=q_ext[h].rearrange("(n p) d -> p n d", p=P))
            kn = nat.tile([P, T_TILES, D], fp32, tag="kn")
            nc.sync.dma_start(out=kn, in_=k_ext[h].rearrange("(n p) d -> p n d", p=P))
            # V1 = [V | ones]. Build it entirely with VectorE ops (copy from a
            # DMA-landed natural tile + memset) so the PV matmul only carries
            # waits on {DVE, ACT} — a third producer engine trips the walrus
            # "too many sync wait commands" limit on LDWEIGHTS.
            vn = nat.tile([P, T_TILES, D], fp32, tag="vn")
            nc.sync.dma_start(out=vn, in_=v_ext[h].rearrange("(n p) d -> p n d", p=P))
            v1 = vpool.tile([P, T_TILES, D + 1], fp32, tag="v1")
            nc.vector.tensor_copy(out=v1[:, :, 0:D], in_=vn)
            nc.vector.memset(v1[:, :, D : D + 1], 1.0)

            qt = qkt.tile([D, S], fp32, tag="qt")
            kt = qkt.tile([D, S], fp32, tag="kt")
            for src, dst, tg in ((qn, qt, "tq"), (kn, kt, "tk")):
                for g in range(4):
                    tp = ps_t.tile([D, 4, P], fp32, tag="qk_t")
                    for j in range(4):
                        nc.tensor.transpose(tp[:, j], src[:, g * 4 + j], identity)
                    nc.vector.tensor_copy(
                        out=dst[:, g * 512 : (g + 1) * 512],
                        in_=tp.rearrange("p a b -> p (a b)"),
                    )

            for qc in range(N_QCHUNKS):
                o_ps = ps_o.tile([D + 1, QCHUNK], fp32, tag="o_acc")
                for t in range(T_TILES):
                    s_ps = ps_s.tile([P, QCHUNK], fp32, tag="scores")
                    for j in range(2):
                        nc.tensor.matmul(
                            s_ps[:, j * 512 : (j + 1) * 512],
                            lhsT=kt[:, t * P : (t + 1) * P],
                            rhs=qt[:, qc * QCHUNK + j * 512 : qc * QCHUNK + (j + 1) * 512],
                            start=True,
                            stop=True,
                        )
                    pt = ptp.tile([P, QCHUNK], fp32, tag="pt")
                    nc.scalar.activation(out=pt, in_=s_ps, func=Exp, scale=SCALE)
                    for j in range(2):
                        nc.tensor.matmul(
                            o_ps[:, j * 512 : (j + 1) * 512],
                            lhsT=v1[:, t],
                            rhs=pt[:, j * 512 : (j + 1) * 512],
                            start=(t == 0),
                            stop=(t == T_TILES - 1),
                        )
                # epilogue: transpose O^T -> [q, 65], normalize, store
                ot_sb = otp.tile([D + 1, QCHUNK], fp32, tag="ot_sb")
                nc.vector.tensor_copy(out=ot_sb, in_=o_ps)
                o_sb = op.tile([P, QCHUNK // P, D], fp32, tag="o_sb")
                for g in range(2):
                    te = ps_e.tile([P, 4, D + 1], fp32, tag="ot_t")
                    for j in range(4):
                        sub = g * 4 + j
                        nc.tensor.transpose(
                            te[:, j],
                            ot_sb[:, sub * P : (sub + 1) * P],
                            identity[0 : D + 1, 0 : D + 1],
                        )
                    rec = rp.tile([P, 4], fp32, tag="rec")
                    nc.vector.reciprocal(out=rec, in_=te[:, :, D])
                    for j in range(4):
                        nc.vector.tensor_scalar_mul(
                            o_sb[:, g * 4 + j], te[:, j, 0:D], rec[:, j : j + 1]
                        )
                nc.sync.dma_start(
                    out=out_ext[h, qc * QCHUNK : (qc + 1) * QCHUNK].rearrange(
                        "(n p) d -> p n d", p=P
                    ),
                    in_=o_sb,
                )
    nc.finalize()
    return nc


def _get_nc():
    if "nc" not in _CACHE:
        _CACHE["nc"] = _build_nc()
    return _CACHE["nc"]


def _shard(x):
    x = np.ascontiguousarray(np.asarray(x), dtype=np.float32).reshape(B * H, S, D)
    return [np.ascontiguousarray(x[i * HPC : (i + 1) * HPC]) for i in range(N_CORES)]


def run(q, k, v, trace=False, **kw):
    from concourse.bass_utils import run_bass_kernel_spmd

    qs, ks, vs = _shard(q), _shard(k), _shard(v)
    in_maps = [{"q": qs[i], "k": ks[i], "v": vs[i]} for i in range(N_CORES)]
    res = run_bass_kernel_spmd(
        _get_nc(), in_maps, core_ids=list(range(N_CORES)), trace=trace, **kw
    )
    out = np.concatenate([res.results[i]["out"] for i in range(N_CORES)], axis=0)
    return out.reshape(B, H, S, D), res


def kernel(q, k, v):
    out, _ = run(q, k, v)
    return out


# revision 7
# speedup vs baseline: 11.9923x; 11.9923x over previous
"""Multi-head attention kernel for Trainium2, SPMD across 8 NeuronCores.

Problem: q,k,v [B=2, H=16, S=2048, D=64] f32;
  out = softmax(q @ k^T / sqrt(4)) @ v      (scale quirk: d_k = tensor RANK = 4)

Sharding: 32 (b,h) heads split 4-per-core across 8 cores; forward is fully
data-parallel (no collectives).

Per-core algorithm (flash-attention style, scores kept transposed):
  - Build Q^T, K^T [64, S] in SBUF via TensorE transposes of natural
    [128, 64] tiles (f32 has no DMA-xbar transpose path).
  - For each q-chunk (1024 q): for each t-tile (128 t):
      S^T[t, q] = K^T_tile.T @ Q^T          (matmul, contraction d=64)
      P^T = exp(0.5 * S^T)                  (ScalarE, PSUM -> SBUF, FD=1024)
      O^T[d+1, q] += V1_tile.T @ P^T        (matmul, contraction t=128;
                                             V1 = [V | ones] so row 64 of O^T
                                             accumulates the softmax denom)
  - Epilogue: transpose O^T back to [q, 65] via TensorE, multiply by
    reciprocal(denom) on VectorE, DMA out.

No max-subtraction in the softmax: scaled scores are ~N(0, 4); |s| < ~25 for
these inputs, exp stays well inside f32 range.
"""

import numpy as np

B, H, S, D = 2, 16, 2048, 64
N_CORES = 8
HPC = (B * H) // N_CORES  # heads per core = 4
P = 128
T_TILES = S // P  # 16
QCHUNK = 1024
N_QCHUNKS = S // QCHUNK  # 2
SCALE = 0.5  # 1/sqrt(d_k) with d_k = k.ndim = 4 (faithful to reference)

_CACHE = {}


def _build_nc(reps=1):
    from contextlib import ExitStack

    import concourse.bacc as bacc
    import concourse.mybir as mybir
    import concourse.tile as tile
    from concourse.masks import make_identity

    fp32 = mybir.dt.float32
    Exp = mybir.ActivationFunctionType.Exp

    nc = bacc.Bacc()
    q_ext = nc.declare_dram_parameter("q", [HPC, S, D], fp32, isOutput=False)
    k_ext = nc.declare_dram_parameter("k", [HPC, S, D], fp32, isOutput=False)
    v_ext = nc.declare_dram_parameter("v", [HPC, S, D], fp32, isOutput=False)
    out_ext = nc.declare_dram_parameter("out", [HPC, S, D], fp32, isOutput=True)

    with ExitStack() as ctx:
        tc = ctx.enter_context(tile.TileContext(nc))
        consts = ctx.enter_context(tc.tile_pool(name="consts", bufs=1))
        identity = consts.tile([P, P], fp32)
        make_identity(nc, identity)

        nat = ctx.enter_context(tc.tile_pool(name="nat", bufs=2))
        vpool = ctx.enter_context(tc.tile_pool(name="vpool", bufs=2))
        qkt = ctx.enter_context(tc.tile_pool(name="qkt", bufs=2))
        ptp = ctx.enter_context(tc.tile_pool(name="ptp", bufs=3))
        otp = ctx.enter_context(tc.tile_pool(name="otp", bufs=2))
        op = ctx.enter_context(tc.tile_pool(name="op", bufs=2))
        rp = ctx.enter_context(tc.tile_pool(name="rp", bufs=2))
        # PSUM budget (8 banks of 2KB/partition):
        #   scores [128,1024] x2 bufs = 4 banks, O-acc [65,1024] = 2 banks,
        #   qk-transpose staging 1 bank, epilogue-transpose staging 1 bank.
        ps_s = ctx.enter_context(tc.tile_pool(name="ps_s", bufs=2, space="PSUM"))
        ps_o = ctx.enter_context(tc.tile_pool(name="ps_o", bufs=1, space="PSUM"))
        ps_t = ctx.enter_context(tc.tile_pool(name="ps_t", bufs=1, space="PSUM"))
        ps_e = ctx.enter_context(tc.tile_pool(name="ps_e", bufs=1, space="PSUM"))

        for h in [h for _ in range(reps) for h in range(HPC)]:
            qn = nat.tile([P, T_TILES, D], fp32, tag="qn")
            nc.sync.dma_start(out=qn, in_=q_ext[h].rearrange("(n p) d -> p n d", p=P))
            kn = nat.tile([P, T_TILES, D], fp32, tag="kn")
            nc.sync.dma_start(out=kn, in_=k_ext[h].rearrange("(n p) d -> p n d", p=P))
            # V1 = [V | ones]. Build it entirely with VectorE ops (copy from a
            # DMA-landed natural tile + memset) so the PV matmul only carries
            # waits on {DVE, ACT} — a third producer engine trips the walrus
            # "too many sync wait commands" limit on LDWEIGHTS.
            vn = nat.tile([P, T_TILES, D], fp32, tag="vn")
            nc.sync.dma_start(out=vn, in_=v_ext[h].rearrange("(n p) d -> p n d", p=P))
            v1 = vpool.tile([P, T_TILES, D + 1], fp32, tag="v1")
            nc.vector.tensor_copy(out=v1[:, :, 0:D], in_=vn)
            nc.vector.memset(v1[:, :, D : D + 1], 1.0)

            qt = qkt.tile([D, S], fp32, tag="qt")
            kt = qkt.tile([D, S], fp32, tag="kt")
            for src, dst, tg in ((qn, qt, "tq"), (kn, kt, "tk")):
                for g in range(4):
                    tp = ps_t.tile([D, 4, P], fp32, tag="qk_t")
                    for j in range(4):
                        nc.tensor.transpose(tp[:, j], src[:, g * 4 + j], identity)
                    nc.vector.tensor_copy(
                        out=dst[:, g * 512 : (g + 1) * 512],
                        in_=tp.rearrange("p a b -> p (a b)"),
                    )

            for qc in range(N_QCHUNKS):
                o_ps = ps_o.tile([D + 1, QCHUNK], fp32, tag="o_acc")
                for t in range(T_TILES):
                    s_ps = ps_s.tile([P, QCHUNK], fp32, tag="scores")
                    for j in range(2):
                        nc.tensor.matmul(
                            s_ps[:, j * 512 : (j + 1) * 512],
                            lhsT=kt[:, t * P : (t + 1) * P],
                            rhs=qt[:, qc * QCHUNK + j * 512 : qc * QCHUNK + (j + 1) * 512],
                            start=True,
                            stop=True,
                        )
                    pt = ptp.tile([P, QCHUNK], fp32, tag="pt")
                    nc.scalar.activation(out=pt, in_=s_ps, func=Exp, scale=SCALE)
                    for j in range(2):
                        nc.tensor.matmul(
                            o_ps[:, j * 512 : (j + 1) * 512],
                            lhsT=v1[:, t],
                            rhs=pt[:, j * 512 : (j + 1) * 512],
                            start=(t == 0),
                            stop=(t == T_TILES - 1),
                        )
                # epilogue: transpose O^T -> [q, 65], normalize, store
                ot_sb = otp.tile([D + 1, QCHUNK], fp32, tag="ot_sb")
                nc.vector.tensor_copy(out=ot_sb, in_=o_ps)
                o_sb = op.tile([P, QCHUNK // P, D], fp32, tag="o_sb")
                for g in range(2):
                    te = ps_e.tile([P, 4, D + 1], fp32, tag="ot_t")
                    for j in range(4):
                        sub = g * 4 + j
                        nc.tensor.transpose(
                            te[:, j],
                            ot_sb[:, sub * P : (sub + 1) * P],
                            identity[0 : D + 1, 0 : D + 1],
                        )
                    rec = rp.tile([P, 4], fp32, tag="rec")
                    nc.vector.reciprocal(out=rec, in_=te[:, :, D])
                    for j in range(4):
                        nc.vector.tensor_scalar_mul(
                            o_sb[:, g * 4 + j], te[:, j, 0:D], rec[:, j : j + 1]
                        )
                nc.sync.dma_start(
                    out=out_ext[h, qc * QCHUNK : (qc + 1) * QCHUNK].rearrange(
                        "(n p) d -> p n d", p=P
                    ),
                    in_=o_sb,
                )
    nc.finalize()
    return nc


def _get_nc(reps=1):
    key = f"nc{reps}"
    if key not in _CACHE:
        _CACHE[key] = _build_nc(reps)
    return _CACHE[key]


def _shard(x):
    x = np.ascontiguousarray(np.asarray(x), dtype=np.float32).reshape(B * H, S, D)
    return [np.ascontiguousarray(x[i * HPC : (i + 1) * HPC]) for i in range(N_CORES)]


def run(q, k, v, trace=False, **kw):
    from concourse.bass_utils import run_bass_kernel_spmd

    qs, ks, vs = _shard(q), _shard(k), _shard(v)
    in_maps = [{"q": qs[i], "k": ks[i], "v": vs[i]} for i in range(N_CORES)]
    res = run_bass_kernel_spmd(
        _get_nc(), in_maps, core_ids=list(range(N_CORES)), trace=trace, **kw
    )
    out = np.concatenate([res.results[i]["out"] for i in range(N_CORES)], axis=0)
    return out.reshape(B, H, S, D), res


def kernel(q, k, v):
    out, _ = run(q, k, v)
    return out


# revision 10
# speedup vs baseline: 25.0298x; 2.0872x over previous
"""Multi-head attention kernel for Trainium2, SPMD across 8 NeuronCores.

Problem: q,k,v [B=2, H=16, S=2048, D=64] f32;
  out = softmax(q @ k^T / sqrt(4)) @ v      (scale quirk: d_k = tensor RANK = 4)

Sharding: 32 (b,h) heads split 4-per-core across 8 cores; forward is fully
data-parallel (no collectives).

Per-core algorithm (flash-attention style, scores kept transposed):
  - Build Q^T, K^T [64, S] in SBUF via TensorE transposes of natural
    [128, 64] tiles (f32 has no DMA-xbar transpose path).
  - For each q-chunk (1024 q): for each t-tile (128 t):
      S^T[t, q] = K^T_tile.T @ Q^T          (matmul, contraction d=64)
      P^T = exp(0.5 * S^T)                  (ScalarE, PSUM -> SBUF, FD=1024)
      O^T[d+2, q] += V1_tile.T @ P^T        (matmul, contraction t=128;
                                             V1 = [V | ones | zeros]; row 64 of
                                             O^T accumulates the softmax denom,
                                             row 65 pads free-dim even for f32r)
  - Epilogue: transpose O^T back to [q, 66] via TensorE, multiply by
    reciprocal(denom) on VectorE, DMA out.

The two big matmuls run with operands bitcast to float32r (TF32-style fast
fp32: 1 cycle/row instead of 4 when the moving free dim >= 256). fp32r ISA
restrictions: even innermost free counts on src/dst, 8B-aligned dst offsets,
dst start_partition 0 — hence the V1 padding to 66.

No max-subtraction in the softmax: scaled scores are ~N(0, 4); |s| < ~25 for
these inputs, exp stays well inside f32 range.
"""

import numpy as np

B, H, S, D = 2, 16, 2048, 64
N_CORES = 8
HPC = (B * H) // N_CORES  # heads per core = 4
P = 128
T_TILES = S // P  # 16
QCHUNK = 1024
N_QCHUNKS = S // QCHUNK  # 2
VE = D + 2  # V1 columns: 64 data + 1 ones (denominator) + 1 zero pad
SCALE = 0.5  # 1/sqrt(d_k) with d_k = k.ndim = 4 (faithful to reference)

_CACHE = {}


def _build_nc(reps=1):
    from contextlib import ExitStack

    import concourse.bacc as bacc
    import concourse.mybir as mybir
    import concourse.tile as tile
    from concourse.masks import make_identity

    fp32 = mybir.dt.float32
    fp32r = mybir.dt.float32r
    Exp = mybir.ActivationFunctionType.Exp

    nc = bacc.Bacc()
    q_ext = nc.declare_dram_parameter("q", [HPC, S, D], fp32, isOutput=False)
    k_ext = nc.declare_dram_parameter("k", [HPC, S, D], fp32, isOutput=False)
    v_ext = nc.declare_dram_parameter("v", [HPC, S, D], fp32, isOutput=False)
    out_ext = nc.declare_dram_parameter("out", [HPC, S, D], fp32, isOutput=True)

    with ExitStack() as ctx:
        tc = ctx.enter_context(tile.TileContext(nc))
        consts = ctx.enter_context(tc.tile_pool(name="consts", bufs=1))
        identity = consts.tile([P, P], fp32)
        make_identity(nc, identity)

        nat = ctx.enter_context(tc.tile_pool(name="nat", bufs=2))
        vpool = ctx.enter_context(tc.tile_pool(name="vpool", bufs=2))
        qkt = ctx.enter_context(tc.tile_pool(name="qkt", bufs=2))
        ptp = ctx.enter_context(tc.tile_pool(name="ptp", bufs=3))
        otp = ctx.enter_context(tc.tile_pool(name="otp", bufs=2))
        op = ctx.enter_context(tc.tile_pool(name="op", bufs=2))
        rp = ctx.enter_context(tc.tile_pool(name="rp", bufs=2))
        # PSUM budget (8 banks of 2KB/partition):
        #   scores [128,1024] x2 bufs = 4 banks, O-acc [66,1024] = 2 banks,
        #   qk-transpose staging 1 bank, epilogue-transpose staging 1 bank.
        ps_s = ctx.enter_context(tc.tile_pool(name="ps_s", bufs=2, space="PSUM"))
        ps_o = ctx.enter_context(tc.tile_pool(name="ps_o", bufs=1, space="PSUM"))
        ps_t = ctx.enter_context(tc.tile_pool(name="ps_t", bufs=1, space="PSUM"))
        ps_e = ctx.enter_context(tc.tile_pool(name="ps_e", bufs=1, space="PSUM"))

        for h in [h for _ in range(reps) for h in range(HPC)]:
            qn = nat.tile([P, T_TILES, D], fp32, tag="qn")
            nc.sync.dma_start(out=qn, in_=q_ext[h].rearrange("(n p) d -> p n d", p=P))
            kn = nat.tile([P, T_TILES, D], fp32, tag="kn")
            nc.sync.dma_start(out=kn, in_=k_ext[h].rearrange("(n p) d -> p n d", p=P))
            # V1 = [V | ones | 0]. Built entirely with VectorE ops (copy from a
            # DMA-landed natural tile + memset) so the PV matmul only carries
            # waits on {DVE, ACT} — a third producer engine trips the walrus
            # "too many sync wait commands" limit on LDWEIGHTS.
            vn = nat.tile([P, T_TILES, D], fp32, tag="vn")
            nc.sync.dma_start(out=vn, in_=v_ext[h].rearrange("(n p) d -> p n d", p=P))
            v1 = vpool.tile([P, T_TILES, VE], fp32r, tag="v1")
            nc.vector.tensor_copy(out=v1[:, :, 0:D], in_=vn)
            # both pad columns = 1.0 (col D is the denominator ones; col D+1 is
            # an even-count pad whose output row is simply ignored). memset
            # can't write f32r, so use in*0 + 1 on VectorE instead.
            nc.vector.tensor_scalar(
                out=v1[:, :, D:VE],
                in0=vn[:, :, 0:2],
                scalar1=0.0,
                scalar2=1.0,
                op0=mybir.AluOpType.mult,
                op1=mybir.AluOpType.add,
            )

            qt = qkt.tile([D, S], fp32r, tag="qt")
            kt = qkt.tile([D, S], fp32r, tag="kt")
            for src, dst in ((qn, qt), (kn, kt)):
                for g in range(4):
                    tp = ps_t.tile([D, 4, P], fp32, tag="qk_t")
                    for j in range(4):
                        nc.tensor.transpose(tp[:, j], src[:, g * 4 + j], identity)
                    nc.vector.tensor_copy(
                        out=dst[:, g * 512 : (g + 1) * 512],
                        in_=tp.rearrange("p a b -> p (a b)"),
                    )

            for qc in range(N_QCHUNKS):
                o_ps = ps_o.tile([VE, QCHUNK], fp32, tag="o_acc")
                for t in range(T_TILES):
                    s_ps = ps_s.tile([P, QCHUNK], fp32, tag="scores")
                    for j in range(2):
                        nc.tensor.matmul(
                            s_ps[:, j * 512 : (j + 1) * 512],
                            lhsT=kt[:, t * P : (t + 1) * P],
                            rhs=qt[
                                :,
                                qc * QCHUNK + j * 512 : qc * QCHUNK + (j + 1) * 512,
                            ],
                            start=True,
                            stop=True,
                        )
                    pt = ptp.tile([P, QCHUNK], fp32r, tag="pt")
                    nc.scalar.activation(out=pt, in_=s_ps, func=Exp, scale=SCALE)
                    for j in range(2):
                        nc.tensor.matmul(
                            o_ps[:, j * 512 : (j + 1) * 512],
                            lhsT=v1[:, t],
                            rhs=pt[:, j * 512 : (j + 1) * 512],
                            start=(t == 0),
                            stop=(t == T_TILES - 1),
                        )
                # epilogue: transpose O^T -> [q, 66], normalize, store
                ot_sb = otp.tile([VE, QCHUNK], fp32, tag="ot_sb")
                nc.vector.tensor_copy(out=ot_sb, in_=o_ps)
                o_sb = op.tile([P, QCHUNK // P, D], fp32, tag="o_sb")
                for g in range(2):
                    te = ps_e.tile([P, 4, VE], fp32, tag="ot_t")
                    for j in range(4):
                        sub = g * 4 + j
                        nc.tensor.transpose(
                            te[:, j],
                            ot_sb[:, sub * P : (sub + 1) * P],
                            identity[0:VE, 0:VE],
                        )
                    rec = rp.tile([P, 4], fp32, tag="rec")
                    nc.vector.reciprocal(out=rec, in_=te[:, :, D])
                    for j in range(4):
                        nc.vector.tensor_scalar_mul(
                            o_sb[:, g * 4 + j], te[:, j, 0:D], rec[:, j : j + 1]
                        )
                nc.sync.dma_start(
                    out=out_ext[h, qc * QCHUNK : (qc + 1) * QCHUNK].rearrange(
                        "(n p) d -> p n d", p=P
                    ),
                    in_=o_sb,
                )
    nc.finalize()
    return nc


def _get_nc(reps=1):
    key = f"nc{reps}"
    if key not in _CACHE:
        _CACHE[key] = _build_nc(reps)
    return _CACHE[key]


def _shard(x):
    x = np.ascontiguousarray(np.asarray(x), dtype=np.float32).reshape(B * H, S, D)
    return [np.ascontiguousarray(x[i * HPC : (i + 1) * HPC]) for i in range(N_CORES)]


def run(q, k, v, trace=False, **kw):
    from concourse.bass_utils import run_bass_kernel_spmd

    qs, ks, vs = _shard(q), _shard(k), _shard(v)
    in_maps = [{"q": qs[i], "k": ks[i], "v": vs[i]} for i in range(N_CORES)]
    res = run_bass_kernel_spmd(
        _get_nc(), in_maps, core_ids=list(range(N_CORES)), trace=trace, **kw
    )
    out = np.concatenate([res.results[i]["out"] for i in range(N_CORES)], axis=0)
    return out.reshape(B, H, S, D), res


def kernel(q, k, v):
    out, _ = run(q, k, v)
    return out


# revision 17
# speedup vs baseline: 25.6312x; 1.0240x over previous
"""Multi-head attention kernel for Trainium2, SPMD across 8 NeuronCores.

Problem: q,k,v [B=2, H=16, S=2048, D=64] f32;
  out = softmax(q @ k^T / sqrt(4)) @ v      (scale quirk: d_k = tensor RANK = 4)

Sharding: 32 (b,h) heads split 4-per-core across 8 cores; forward is fully
data-parallel (no collectives).

Per-core algorithm (flash-attention style, scores kept transposed, heads
processed in PAIRS to pack the 128-row PE array):
  - Build paired Q^T, K^T [128, S]: partitions 0-63 hold head A's [d, s],
    partitions 64-127 head B's, via TensorE transposes (no DMA-xbar transpose
    path for 4-byte dtypes).
  - For each q-chunk (512 q): for each t-tile (128 t):
      S^T_A[t, q] = K^T_A.T @ Q^T_A   (K=64, PE row groups 0-1)
      S^T_B[t, q] = K^T_B.T @ Q^T_B   (K=64, PE row groups 2-3 — runs
                                       CONCURRENT with A via tile_position
                                       derived from base_partition 64)
      P^T_{A,B} = exp(0.5 * S^T)      (ScalarE, PSUM -> SBUF, FD=512)
      O^T[d+2, q] += V1_tile.T @ P^T  (matmul, contraction t=128;
                                       V1 = [V | ones | ones]; row 64 of O^T
                                       accumulates the softmax denominator)
    PV matmuls are software-pipelined one t-tile behind the scores matmuls so
    the in-order PE queue never stalls waiting on ScalarE's exp.
  - Epilogue: transpose O^T back to [q, 66] via TensorE, multiply by
    reciprocal(denominator) on VectorE, DMA out.

The big matmuls run in float32r (TF32-style fast fp32: ~1 cycle/row instead
of 4). fp32r ISA restrictions: even innermost free counts, 8B-aligned dst
offsets, dst start_partition 0 — hence V1 padded to 66 columns.

No max-subtraction in the softmax: scaled scores are ~N(0, 4); |s| < ~25 for
these inputs, exp stays well inside f32 range.
"""

import numpy as np

B, H, S, D = 2, 16, 2048, 64
N_CORES = 8
HPC = (B * H) // N_CORES  # heads per core = 4
P = 128
T_TILES = S // P  # 16
QCHUNK = 512
N_QCHUNKS = S // QCHUNK  # 4
VE = D + 2  # V1 columns: 64 data + 1 ones (denominator) + 1 pad
SCALE = 0.5  # 1/sqrt(d_k) with d_k = k.ndim = 4 (faithful to reference)

_CACHE = {}


def _build_nc(reps=1, pack=True):
    from contextlib import ExitStack

    import concourse.bacc as bacc
    import concourse.mybir as mybir
    import concourse.tile as tile
    from concourse.masks import make_identity

    fp32 = mybir.dt.float32
    fp32r = mybir.dt.float32r
    Exp = mybir.ActivationFunctionType.Exp

    nc = bacc.Bacc()
    q_ext = nc.declare_dram_parameter("q", [HPC, S, D], fp32, isOutput=False)
    k_ext = nc.declare_dram_parameter("k", [HPC, S, D], fp32, isOutput=False)
    v_ext = nc.declare_dram_parameter("v", [HPC, S, D], fp32, isOutput=False)
    out_ext = nc.declare_dram_parameter("out", [HPC, S, D], fp32, isOutput=True)

    with ExitStack() as ctx:
        tc = ctx.enter_context(tile.TileContext(nc))
        consts = ctx.enter_context(tc.tile_pool(name="consts", bufs=1))
        identity = consts.tile([P, P], fp32)
        make_identity(nc, identity)

        nat = ctx.enter_context(tc.tile_pool(name="nat", bufs=2))
        vpool = ctx.enter_context(tc.tile_pool(name="vpool", bufs=2))
        qkt = ctx.enter_context(tc.tile_pool(name="qkt", bufs=2))
        ptp = ctx.enter_context(tc.tile_pool(name="ptp", bufs=6))
        otp = ctx.enter_context(tc.tile_pool(name="otp", bufs=2))
        op = ctx.enter_context(tc.tile_pool(name="op", bufs=2))
        rp = ctx.enter_context(tc.tile_pool(name="rp", bufs=2))
        # PSUM budget (8 banks of 2KB/partition):
        #   scores A/B [128,512] x2 bufs each = 4 banks, O-acc A/B = 2 banks,
        #   qk-transpose staging 1 bank, epilogue-transpose staging 1 bank.
        ps_s = ctx.enter_context(tc.tile_pool(name="ps_s", bufs=2, space="PSUM"))
        ps_o = ctx.enter_context(tc.tile_pool(name="ps_o", bufs=1, space="PSUM"))
        ps_t = ctx.enter_context(tc.tile_pool(name="ps_t", bufs=1, space="PSUM"))
        ps_e = ctx.enter_context(tc.tile_pool(name="ps_e", bufs=1, space="PSUM"))

        for pair in [pr for _ in range(reps) for pr in range(HPC // 2)]:
            hA, hB = 2 * pair, 2 * pair + 1
            # natural loads for both heads of the pair
            qn = nat.tile([P, 2, T_TILES, D], fp32, tag="qn")
            kn = nat.tile([P, 2, T_TILES, D], fp32, tag="kn")
            vn = nat.tile([P, 2, T_TILES, D], fp32, tag="vn")
            for i, hh in enumerate((hA, hB)):
                nc.sync.dma_start(
                    out=qn[:, i], in_=q_ext[hh].rearrange("(n p) d -> p n d", p=P)
                )
                nc.sync.dma_start(
                    out=kn[:, i], in_=k_ext[hh].rearrange("(n p) d -> p n d", p=P)
                )
                nc.sync.dma_start(
                    out=vn[:, i], in_=v_ext[hh].rearrange("(n p) d -> p n d", p=P)
                )
            # V1 = [V | ones | ones] per head, built entirely on VectorE (DVE)
            # so the PV matmul only waits on {DVE, ACT}: a third producer
            # engine trips walrus' sync-wait limit on LDWEIGHTS.
            v1s = []
            for i in range(2):
                v1 = vpool.tile([P, T_TILES, VE], fp32r, tag=f"v1{i}")
                nc.vector.tensor_copy(out=v1[:, :, 0:D], in_=vn[:, i])
                nc.vector.tensor_scalar(
                    out=v1[:, :, D:VE],
                    in0=vn[:, i, :, 0:2],
                    scalar1=0.0,
                    scalar2=1.0,
                    op0=mybir.AluOpType.mult,
                    op1=mybir.AluOpType.add,
                )
                v1s.append(v1)

            # Paired transposed layouts: partitions 0-63 head A, 64-127 head
            # B. Transpose outputs must land at PSUM partition 0, and VectorE
            # cannot shift partitions, so head B goes through a base-0 staging
            # SBUF tile and an SBUF->SBUF DMA (which can cross partitions).
            if pack:
                qt = qkt.tile([P, S], fp32r, tag="qt")
                kt = qkt.tile([P, S], fp32r, tag="kt")
                qts, kts = (qt[0:D],), (kt[0:D],)
            else:
                qtA = qkt.tile([D, S], fp32r, tag="qtA")
                qtB = qkt.tile([D, S], fp32r, tag="qtB")
                ktA = qkt.tile([D, S], fp32r, tag="ktA")
                ktB = qkt.tile([D, S], fp32r, tag="ktB")
                qts, kts = (qtA, qtB), (ktA, ktB)
            stgs = {}
            if pack:
                for nm in ("q", "k"):
                    stgs[nm] = qkt.tile([D, S], fp32r, tag=f"stg{nm}", name=f"stg{nm}")
                qts = (qts[0], stgs["q"])
                kts = (kts[0], stgs["k"])
            for g in range(4):
                for srcn, dsts in ((qn, qts), (kn, kts)):
                    for i in range(2):
                        tp = ps_t.tile([D, 4, P], fp32, tag="qk_t")
                        for j in range(4):
                            nc.tensor.transpose(
                                tp[:, j],
                                srcn[:, i, g * 4 + j],
                                identity,
                            )
                        nc.vector.tensor_copy(
                            out=dsts[i][:, g * 512 : (g + 1) * 512],
                            in_=tp.rearrange("p a b -> p (a b)"),
                        )
            if pack:
                nc.sync.dma_start(out=qt[D:P, :], in_=stgs["q"][:])
                nc.sync.dma_start(out=kt[D:P, :], in_=stgs["k"][:])
                qts = (qt[0:D], qt[D:P])
                kts = (kt[0:D], kt[D:P])

            for qc in range(N_QCHUNKS):
                o_psA = ps_o.tile([VE, QCHUNK], fp32, tag="o_accA")
                o_psB = ps_o.tile([VE, QCHUNK], fp32, tag="o_accB")
                o_pss = (o_psA, o_psB)
                pts = {}

                def pv(t):
                    for i in range(2):
                        nc.tensor.matmul(
                            o_pss[i],
                            lhsT=v1s[i][:, t],
                            rhs=pts[(t, i)],
                            start=(t == 0),
                            stop=(t == T_TILES - 1),
                        )

                qsl = slice(qc * QCHUNK, (qc + 1) * QCHUNK)
                for t in range(T_TILES):
                    tsl = slice(t * P, (t + 1) * P)
                    s_psA = ps_s.tile([P, QCHUNK], fp32, tag="scoresA")
                    s_psB = ps_s.tile([P, QCHUNK], fp32, tag="scoresB")
                    # A on row groups 0-1, B on 2-3: concurrent on the PE
                    nc.tensor.matmul(
                        s_psA, lhsT=kts[0][:, tsl], rhs=qts[0][:, qsl],
                        start=True, stop=True,
                    )
                    nc.tensor.matmul(
                        s_psB, lhsT=kts[1][:, tsl], rhs=qts[1][:, qsl],
                        start=True, stop=True,
                    )
                    if t > 0:
                        pv(t - 1)
                    for i, s_ps in enumerate((s_psA, s_psB)):
                        pt = ptp.tile([P, QCHUNK], fp32r, tag="pt")
                        pts[(t, i)] = pt
                        nc.scalar.activation(out=pt, in_=s_ps, func=Exp, scale=SCALE)
                pv(T_TILES - 1)

                # epilogue per head: transpose O^T -> [q, 66], normalize, store
                for i, hh in enumerate((hA, hB)):
                    ot_sb = otp.tile([VE, QCHUNK], fp32, tag="ot_sb")
                    nc.vector.tensor_copy(out=ot_sb, in_=o_pss[i])
                    o_sb = op.tile([P, QCHUNK // P, D], fp32, tag="o_sb")
                    te = ps_e.tile([P, 4, VE], fp32, tag="ot_t")
                    for j in range(4):
                        nc.tensor.transpose(
                            te[:, j],
                            ot_sb[:, j * P : (j + 1) * P],
                            identity[0:VE, 0:VE],
                        )
                    rec = rp.tile([P, 4], fp32, tag="rec")
                    nc.vector.reciprocal(out=rec, in_=te[:, :, D])
                    for j in range(4):
                        nc.vector.tensor_scalar_mul(
                            o_sb[:, j], te[:, j, 0:D], rec[:, j : j + 1]
                        )
                    nc.sync.dma_start(
                        out=out_ext[hh, qc * QCHUNK : (qc + 1) * QCHUNK].rearrange(
                            "(n p) d -> p n d", p=P
                        ),
                        in_=o_sb,
                    )
    nc.finalize()
    return nc


def _get_nc(reps=1, pack=True):
    key = f"nc{reps}p{pack}"
    if key not in _CACHE:
        _CACHE[key] = _build_nc(reps, pack=pack)
    return _CACHE[key]


def _shard(x):
    x = np.ascontiguousarray(np.asarray(x), dtype=np.float32).reshape(B * H, S, D)
    return [np.ascontiguousarray(x[i * HPC : (i + 1) * HPC]) for i in range(N_CORES)]


def run(q, k, v, trace=False, **kw):
    from concourse.bass_utils import run_bass_kernel_spmd

    qs, ks, vs = _shard(q), _shard(k), _shard(v)
    in_maps = [{"q": qs[i], "k": ks[i], "v": vs[i]} for i in range(N_CORES)]
    res = run_bass_kernel_spmd(
        _get_nc(), in_maps, core_ids=list(range(N_CORES)), trace=trace, **kw
    )
    out = np.concatenate([res.results[i]["out"] for i in range(N_CORES)], axis=0)
    return out.reshape(B, H, S, D), res


def kernel(q, k, v):
    out, _ = run(q, k, v)
    return out


# revision 18
# speedup vs baseline: 26.1296x; 1.0194x over previous
"""Multi-head attention kernel for Trainium2, SPMD across 8 NeuronCores.

Problem: q,k,v [B=2, H=16, S=2048, D=64] f32;
  out = softmax(q @ k^T / sqrt(4)) @ v      (scale quirk: d_k = tensor RANK = 4)

Sharding: 32 (b,h) heads split 4-per-core across 8 cores; forward is fully
data-parallel (no collectives).

Per-core algorithm (flash-attention style, scores kept transposed, heads
processed in PAIRS to pack the 128-row PE array):
  - Build paired Q^T, K^T [128, S]: partitions 0-63 hold head A's [d, s],
    partitions 64-127 head B's, via TensorE transposes (no DMA-xbar transpose
    path for 4-byte dtypes).
  - For each q-chunk (512 q): for each t-tile (128 t):
      S^T_A[t, q] = K^T_A.T @ Q^T_A   (K=64, PE row groups 0-1)
      S^T_B[t, q] = K^T_B.T @ Q^T_B   (K=64, PE row groups 2-3 — runs
                                       CONCURRENT with A via tile_position
                                       derived from base_partition 64)
      P^T_{A,B} = exp(0.5 * S^T)      (ScalarE, PSUM -> SBUF, FD=512)
      O^T[d+2, q] += V1_tile.T @ P^T  (matmul, contraction t=128;
                                       V1 = [V | ones | ones]; row 64 of O^T
                                       accumulates the softmax denominator)
    PV matmuls are software-pipelined one t-tile behind the scores matmuls so
    the in-order PE queue never stalls waiting on ScalarE's exp.
  - Epilogue: transpose O^T back to [q, 66] via TensorE, multiply by
    reciprocal(denominator) on VectorE, DMA out.

The big matmuls run in float32r (TF32-style fast fp32: ~1 cycle/row instead
of 4). fp32r ISA restrictions: even innermost free counts, 8B-aligned dst
offsets, dst start_partition 0 — hence V1 padded to 66 columns.

No max-subtraction in the softmax: scaled scores are ~N(0, 4); |s| < ~25 for
these inputs, exp stays well inside f32 range.
"""

import numpy as np

B, H, S, D = 2, 16, 2048, 64
N_CORES = 8
HPC = (B * H) // N_CORES  # heads per core = 4
P = 128
T_TILES = S // P  # 16
QCHUNK = 512
N_QCHUNKS = S // QCHUNK  # 4
VE = D + 2  # V1 columns: 64 data + 1 ones (denominator) + 1 pad
SCALE = 0.5  # 1/sqrt(d_k) with d_k = k.ndim = 4 (faithful to reference)

_CACHE = {}


def _build_nc(reps=1, pack=False):
    from contextlib import ExitStack

    import concourse.bacc as bacc
    import concourse.mybir as mybir
    import concourse.tile as tile
    from concourse.masks import make_identity

    fp32 = mybir.dt.float32
    fp32r = mybir.dt.float32r
    Exp = mybir.ActivationFunctionType.Exp

    nc = bacc.Bacc()
    q_ext = nc.declare_dram_parameter("q", [HPC, S, D], fp32, isOutput=False)
    k_ext = nc.declare_dram_parameter("k", [HPC, S, D], fp32, isOutput=False)
    v_ext = nc.declare_dram_parameter("v", [HPC, S, D], fp32, isOutput=False)
    out_ext = nc.declare_dram_parameter("out", [HPC, S, D], fp32, isOutput=True)

    with ExitStack() as ctx:
        tc = ctx.enter_context(tile.TileContext(nc))
        consts = ctx.enter_context(tc.tile_pool(name="consts", bufs=1))
        identity = consts.tile([P, P], fp32)
        make_identity(nc, identity)

        nat = ctx.enter_context(tc.tile_pool(name="nat", bufs=2))
        vpool = ctx.enter_context(tc.tile_pool(name="vpool", bufs=2))
        qkt = ctx.enter_context(tc.tile_pool(name="qkt", bufs=2))
        ptp = ctx.enter_context(tc.tile_pool(name="ptp", bufs=6))
        otp = ctx.enter_context(tc.tile_pool(name="otp", bufs=2))
        op = ctx.enter_context(tc.tile_pool(name="op", bufs=2))
        rp = ctx.enter_context(tc.tile_pool(name="rp", bufs=2))
        # PSUM budget (8 banks of 2KB/partition):
        #   scores A/B [128,512] x2 bufs each = 4 banks, O-acc A/B = 2 banks,
        #   qk-transpose staging 1 bank, epilogue-transpose staging 1 bank.
        ps_s = ctx.enter_context(tc.tile_pool(name="ps_s", bufs=2, space="PSUM"))
        ps_o = ctx.enter_context(tc.tile_pool(name="ps_o", bufs=1, space="PSUM"))
        ps_t = ctx.enter_context(tc.tile_pool(name="ps_t", bufs=1, space="PSUM"))
        ps_e = ctx.enter_context(tc.tile_pool(name="ps_e", bufs=1, space="PSUM"))

        def prep_pair(hA, hB):
            """Emit DMA loads + V1 builds; return (state, transpose work units).

            The transpose units are emitted by the caller interleaved into the
            previous pair's ACT-bound main loop so the PE does them in slack
            cycles instead of a serial phase where ScalarE would idle.
            """
            qn = nat.tile([P, 2, T_TILES, D], fp32, tag="qn", name="qn")
            kn = nat.tile([P, 2, T_TILES, D], fp32, tag="kn", name="kn")
            vn = nat.tile([P, 2, T_TILES, D], fp32, tag="vn", name="vn")
            for i, hh in enumerate((hA, hB)):
                nc.sync.dma_start(
                    out=qn[:, i], in_=q_ext[hh].rearrange("(n p) d -> p n d", p=P)
                )
                nc.sync.dma_start(
                    out=kn[:, i], in_=k_ext[hh].rearrange("(n p) d -> p n d", p=P)
                )
                nc.sync.dma_start(
                    out=vn[:, i], in_=v_ext[hh].rearrange("(n p) d -> p n d", p=P)
                )
            # V1 = [V | ones | ones] per head, built entirely on VectorE (DVE)
            # so the PV matmul only waits on {DVE, ACT}: a third producer
            # engine trips walrus' sync-wait limit on LDWEIGHTS.
            v1s = []
            for i in range(2):
                v1 = vpool.tile([P, T_TILES, VE], fp32r, tag=f"v1{i}", name="v1")
                nc.vector.tensor_copy(out=v1[:, :, 0:D], in_=vn[:, i])
                nc.vector.tensor_scalar(
                    out=v1[:, :, D:VE],
                    in0=vn[:, i, :, 0:2],
                    scalar1=0.0,
                    scalar2=1.0,
                    op0=mybir.AluOpType.mult,
                    op1=mybir.AluOpType.add,
                )
                v1s.append(v1)
            qtA = qkt.tile([D, S], fp32r, tag="qtA", name="qtA")
            qtB = qkt.tile([D, S], fp32r, tag="qtB", name="qtB")
            ktA = qkt.tile([D, S], fp32r, tag="ktA", name="ktA")
            ktB = qkt.tile([D, S], fp32r, tag="ktB", name="ktB")
            qts, kts = (qtA, qtB), (ktA, ktB)

            def unit(g, srcn, dsts, i):
                def emit():
                    tp = ps_t.tile([D, 4, P], fp32, tag="qk_t", name="tp")
                    for j in range(4):
                        nc.tensor.transpose(
                            tp[:, j], srcn[:, i, g * 4 + j], identity
                        )
                    nc.vector.tensor_copy(
                        out=dsts[i][:, g * 512 : (g + 1) * 512],
                        in_=tp.rearrange("p a b -> p (a b)"),
                    )
                return emit

            units = [
                unit(g, srcn, dsts, i)
                for g in range(4)
                for srcn, dsts in ((qn, qts), (kn, kts))
                for i in range(2)
            ]
            return (qts, kts, v1s), units

        pair_seq = [
            (2 * pr, 2 * pr + 1) for _ in range(reps) for pr in range(HPC // 2)
        ]
        state, units = prep_pair(*pair_seq[0])
        for pi, (hA, hB) in enumerate(pair_seq):
            for u in units:  # leftovers not absorbed by the previous main loop
                u()
            qts, kts, v1s = state
            next_units = []
            if pi + 1 < len(pair_seq):
                state, next_units = prep_pair(*pair_seq[pi + 1])
            units = next_units

            for qc in range(N_QCHUNKS):
                o_psA = ps_o.tile([VE, QCHUNK], fp32, tag="o_accA", name="o_psA")
                o_psB = ps_o.tile([VE, QCHUNK], fp32, tag="o_accB", name="o_psB")
                o_pss = (o_psA, o_psB)
                pts = {}

                def pv(t):
                    for i in range(2):
                        nc.tensor.matmul(
                            o_pss[i],
                            lhsT=v1s[i][:, t],
                            rhs=pts[(t, i)],
                            start=(t == 0),
                            stop=(t == T_TILES - 1),
                        )

                qsl = slice(qc * QCHUNK, (qc + 1) * QCHUNK)
                for t in range(T_TILES):
                    # absorb one next-pair transpose unit every 4th iteration
                    if t % 4 == 0 and units:
                        units.pop(0)()
                    tsl = slice(t * P, (t + 1) * P)
                    s_psA = ps_s.tile([P, QCHUNK], fp32, tag="scoresA", name="s_psA")
                    s_psB = ps_s.tile([P, QCHUNK], fp32, tag="scoresB", name="s_psB")
                    # A on row groups 0-1, B on 2-3: concurrent on the PE
                    nc.tensor.matmul(
                        s_psA, lhsT=kts[0][:, tsl], rhs=qts[0][:, qsl],
                        start=True, stop=True,
                    )
                    nc.tensor.matmul(
                        s_psB, lhsT=kts[1][:, tsl], rhs=qts[1][:, qsl],
                        start=True, stop=True,
                    )
                    if t > 0:
                        pv(t - 1)
                    for i, s_ps in enumerate((s_psA, s_psB)):
                        pt = ptp.tile([P, QCHUNK], fp32r, tag="pt")
                        pts[(t, i)] = pt
                        nc.scalar.activation(out=pt, in_=s_ps, func=Exp, scale=SCALE)
                pv(T_TILES - 1)

                # epilogue per head: transpose O^T -> [q, 66], normalize, store
                for i, hh in enumerate((hA, hB)):
                    ot_sb = otp.tile([VE, QCHUNK], fp32, tag="ot_sb")
                    nc.vector.tensor_copy(out=ot_sb, in_=o_pss[i])
                    o_sb = op.tile([P, QCHUNK // P, D], fp32, tag="o_sb")
                    te = ps_e.tile([P, 4, VE], fp32, tag="ot_t")
                    for j in range(4):
                        nc.tensor.transpose(
                            te[:, j],
                            ot_sb[:, j * P : (j + 1) * P],
                            identity[0:VE, 0:VE],
                        )
                    rec = rp.tile([P, 4], fp32, tag="rec")
                    nc.vector.reciprocal(out=rec, in_=te[:, :, D])
                    for j in range(4):
                        nc.vector.tensor_scalar_mul(
                            o_sb[:, j], te[:, j, 0:D], rec[:, j : j + 1]
                        )
                    nc.sync.dma_start(
                        out=out_ext[hh, qc * QCHUNK : (qc + 1) * QCHUNK].rearrange(
                            "(n p) d -> p n d", p=P
                        ),
                        in_=o_sb,
                    )
    nc.finalize()
    return nc


def _get_nc(reps=1, pack=False):
    key = f"nc{reps}p{pack}"
    if key not in _CACHE:
        _CACHE[key] = _build_nc(reps, pack=pack)
    return _CACHE[key]


def _shard(x):
    x = np.ascontiguousarray(np.asarray(x), dtype=np.float32).reshape(B * H, S, D)
    return [np.ascontiguousarray(x[i * HPC : (i + 1) * HPC]) for i in range(N_CORES)]


def run(q, k, v, trace=False, **kw):
    from concourse.bass_utils import run_bass_kernel_spmd

    qs, ks, vs = _shard(q), _shard(k), _shard(v)
    in_maps = [{"q": qs[i], "k": ks[i], "v": vs[i]} for i in range(N_CORES)]
    res = run_bass_kernel_spmd(
        _get_nc(), in_maps, core_ids=list(range(N_CORES)), trace=trace, **kw
    )
    out = np.concatenate([res.results[i]["out"] for i in range(N_CORES)], axis=0)
    return out.reshape(B, H, S, D), res


def kernel(q, k, v):
    out, _ = run(q, k, v)
    return out
